# revision 5
# baseline (speedup 1.0000x reference)
"""Contrastive loss (SimCLR-style) on 8 Trainium2 NeuronCores.

Full inputs in, full output out.  Each core owns a 1024-row block of
feats.  The host pre-normalizes feats (fp32), casts to bf16, and passes
each core the TRANSPOSED layout nfT = nf.T [D=128, cols], rolled so the
core's own block is columns 0..1023.  Because cos = nfT.T @ nfT, the
transposed layout serves as BOTH matmul operands: no on-device norms,
no transposes, no rsqrt pipeline.

Symmetry split: exp(cos/T) is symmetric, so core x only computes its
1024 rows against local column blocks 0..4 (cols 0..5119).  Row sums
over the remaining column blocks 5..7 are recovered from *column* sums
of blocks 1..3 (ones-stationary matmuls accumulated over the 8 row
tiles), shipped to the host, which adds them into the right rows.

The self-similarity diagonal is NOT masked on device: the host knows
exactly what the device computes for z_ii (fp32 dot of the bf16 row
with itself) and subtracts exp(z_ii/T) from the assembled row sums.

exp is split across two engines: ACT evaluates blocks q=0..2 with its
Exp table (free row-sum accumulation); the DVE evaluates blocks q=3,4
with a bf16 Schraudolph bit-trick -- i16 = z*A + B truncated, bitcast
to bf16 is 2^(z*log2e/T + sigma), with B calibrated so the SUM of the
approximated exps is unbiased (row-sum rel err ~1e-3 << tolerance).
A second DVE pass row-sums both blocks per m-tile in one op.

Engine budget per core: PE does 80 similarity matmuls (5 blocks x 8
m-tiles x 2x512) plus 48 column-sum matmuls; ACT does 24 exp ops; DVE
does 16 Schraudolph ops + 8 row-sum ops + 3 column-sum copies.  Host:
normalize, transpose, positive pairs, diag subtraction, logsumexp.
"""

from contextlib import ExitStack

import numpy as np

N, D, NCORES = 8192, 128, 8
BLK = N // NCORES            # 1024 rows per core
TPB = BLK // 128             # 8 M-tiles (of 128 rows) per core
NQ = 5                       # direct column blocks per core (cols 0..5119)
NQA = 3                      # blocks on ACT (q=0..2); q=3,4 ride the DVE
CSBLKS = 3                   # column-sum blocks (local col blocks 1..3)
COLS = NQ * BLK              # 5120 columns held on device
TEMP = 0.07
EPS = 1e-8
SCH_A = float(np.float32((1.0 / TEMP) * np.log2(np.e) * 128.0))
SCH_B = 16249.0              # calibrated: unbiased sum of bf16-Schraudolph exp
NPARTS = NQA * TPB + TPB     # 24 ACT cols + 8 DVE cols

_CACHE = {}
_AUX = {}
LAST_RESULT = None


def _emit(tc, xt, s_out, cs_out, rep=0):
    import concourse.mybir as mybir

    nc = tc.nc
    f32 = mybir.dt.float32
    bf16 = mybir.dt.bfloat16
    i16 = mybir.dt.int16
    AF = mybir.ActivationFunctionType
    ALU = mybir.AluOpType
    SCALE = 1.0 / TEMP

    with ExitStack() as ctx:
        singles = ctx.enter_context(tc.tile_pool(name=f"singles{rep}", bufs=1))
        esp = ctx.enter_context(tc.tile_pool(name=f"esp{rep}", bufs=TPB))
        e2p = ctx.enter_context(tc.tile_pool(name=f"e2p{rep}", bufs=TPB))
        junkp = ctx.enter_context(tc.tile_pool(name=f"junkp{rep}", bufs=2))

        xbig = singles.tile([128, COLS], bf16, tag="xbig")    # rolled nf^T
        onesb = singles.tile([128, 128], bf16, tag="onesb")
        parts = singles.tile([128, NPARTS], f32, tag="parts")
        colacc = singles.tile([1, CSBLKS * BLK], f32, tag="colacc")

        # block 0 first (stationaries + q=0 moving), rest streams in under
        # the q=0 round's compute
        nc.sync.dma_start(out=xbig[:, 0:BLK], in_=xt[:, 0:BLK])
        nc.sync.dma_start(out=xbig[:, BLK:COLS], in_=xt[:, BLK:COLS])
        nc.vector.memset(onesb[:], 1.0)

        # prime the ACT Exp table at t~0 so the first real exp doesn't pay
        # the table load on the critical path
        warm = singles.tile([128, 1], f32, tag="warm")
        nc.vector.memset(warm[:], 0.0)
        nc.scalar.activation(warm[:], warm[:], AF.Exp)

        with (
            tc.tile_pool(name=f"mpsum{rep}", bufs=3, space="PSUM") as mpsum,
            tc.tile_pool(name=f"cpsum{rep}", bufs=1, space="PSUM") as cpsum,
        ):
            e2tiles = [None] * TPB
            for q in range(NQ):
                do_cs = 1 <= q <= CSBLKS
                etiles = []
                for m in range(TPB):
                    z = mpsum.tile([128, BLK], f32, tag="z")
                    lhsT = xbig[:, m * 128:(m + 1) * 128]
                    c0 = q * BLK
                    nc.tensor.matmul(z[:, 0:512], lhsT, xbig[:, c0:c0 + 512])
                    nc.tensor.matmul(z[:, 512:1024], lhsT,
                                     xbig[:, c0 + 512:c0 + 1024])
                    if q < NQA:
                        # ACT stream: spline exp, free row-sum accumulation
                        e = esp.tile([128, BLK], bf16, tag="e")
                        col = q * TPB + m
                        nc.scalar.activation(e[:], z[:], AF.Exp, scale=SCALE,
                                             accum_out=parts[:, col:col + 1])
                        etiles.append(e)
                    else:
                        # DVE stream: i16 = z*A + B, bitcast bf16 ~ exp(z/T)
                        if q == NQA:
                            e2tiles[m] = e2p.tile([128, 2 * BLK], i16,
                                                  name="e2", tag="e2")
                        half = e2tiles[m][:, (q - NQA) * BLK:
                                          (q - NQA + 1) * BLK]
                        nc.vector.tensor_scalar(half, z[:], SCH_A, SCH_B,
                                                ALU.mult, ALU.add)
                        if q == NQ - 1:
                            # one row-sum op covering both DVE blocks
                            eb = e2tiles[m][:].bitcast(bf16)
                            j = junkp.tile([128, 2 * BLK], bf16, tag="junk")
                            col = NQA * TPB + m
                            nc.vector.tensor_scalar(
                                j[:], eb, 1.0, 0.0, ALU.mult, ALU.add,
                                accum_out=parts[:, col:col + 1])
                        if do_cs:
                            etiles.append(e2tiles[m])
                if do_cs:
                    # column sums of exp accumulated across the 8 M-tiles;
                    # one ones-LDWEIGHTS per round (batched after the e's)
                    cs = cpsum.tile([128, BLK], f32, tag="cs")
                    for m in range(TPB):
                        if q < NQA:
                            eap = etiles[m][:]
                        else:
                            eap = etiles[m][:, 0:BLK].bitcast(bf16)
                        nc.tensor.matmul(cs[:, 0:512], onesb[:],
                                         eap[:, 0:512],
                                         start=(m == 0), stop=(m == TPB - 1),
                                         skip_group_check=True)
                        nc.tensor.matmul(cs[:, 512:1024], onesb[:],
                                         eap[:, 512:1024],
                                         start=(m == 0), stop=(m == TPB - 1),
                                         skip_group_check=True)
                    k = q - 1
                    nc.vector.tensor_copy(colacc[0:1, k * BLK:(k + 1) * BLK],
                                          cs[0:1, :])

        nc.sync.dma_start(out=s_out, in_=parts[:])
        nc.sync.dma_start(out=cs_out, in_=colacc[:])


def declare_io(nc):
    """Declare the kernel's DRAM I/O on `nc`; returns the APs _emit wants."""
    import concourse.mybir as mybir

    f32 = mybir.dt.float32
    bf16 = mybir.dt.bfloat16
    xt_h = nc.dram_tensor("xt", [128, COLS], bf16, kind="ExternalInput")
    s_h = nc.dram_tensor("s_out", [128, NPARTS], f32, kind="ExternalOutput")
    c_h = nc.dram_tensor("cs_out", [1, CSBLKS * BLK], f32,
                         kind="ExternalOutput")
    return (xt_h.ap(), s_h.ap(), c_h.ap())


def _build_nc(repeats=1):
    import concourse.tile as tile
    from concourse import bacc

    nc = bacc.Bacc(
        "TRN2", target_bir_lowering=False, debug=False,
        enable_asserts=False, num_devices=NCORES,
    )
    aps = declare_io(nc)
    with tile.TileContext(nc, trace_sim=False) as tc:
        for rep in range(repeats):
            _emit(tc, *aps, rep=rep)
    nc.compile()
    return nc


def get_nc(repeats=1):
    key = ("nc", repeats)
    if key not in _CACHE:
        _CACHE[key] = _build_nc(repeats)
    return _CACHE[key]


def make_in_maps(feats, label):
    import ml_dtypes

    feats = np.asarray(feats, dtype=np.float32)
    label = np.asarray(label)
    norms = np.sqrt((feats ** 2).sum(axis=1))
    nf = feats / np.maximum(norms, EPS)[:, None]
    nf16 = nf.astype(ml_dtypes.bfloat16)
    nfT = np.ascontiguousarray(nf16.T)                 # [128, 8192]
    nfT2 = np.concatenate([nfT, nfT], axis=1)          # wraparound roll
    in_maps = [
        {"xt": np.ascontiguousarray(nfT2[:, c * BLK:c * BLK + COLS])}
        for c in range(NCORES)
    ]

    # host-side replication of the device's self-similarity term:
    # z_ii = fp32 dot of the bf16 row with itself, e_ii = exp(z_ii * 1/T)
    nf16f = nf16.astype(np.float32)
    ssq = (nf16f ** 2).sum(axis=1, dtype=np.float32)
    _AUX["diag"] = np.exp((ssq * np.float32(1.0 / TEMP)).astype(np.float32)
                          ).astype(np.float64)
    pos_idx = np.argmax(label, axis=1)
    nf64 = nf.astype(np.float64)
    _AUX["pos"] = (nf64 * nf64[pos_idx]).sum(axis=1) / TEMP
    return in_maps


def finish(results):
    """Host epilogue: assemble full row sums from direct row partials and
    symmetric column partials, subtract the exact diagonal, logsumexp,
    subtract positive-pair logits, mean."""
    S = np.zeros(N, dtype=np.float64)
    for x in range(NCORES):
        parts = results[x]["s_out"].astype(np.float64)     # [128, NPARTS]
        sv = parts[:, :NQA * TPB].reshape(128, NQA, TPB).sum(axis=1)
        sv += parts[:, NQA * TPB:NPARTS]                   # DVE q=3,4 sums
        S[x * BLK:(x + 1) * BLK] += sv.T.reshape(-1)       # row = m*128+p
        cs = results[x]["cs_out"].astype(np.float64).reshape(CSBLKS, BLK)
        for k in range(1, CSBLKS + 1):
            tgt = ((x + k) % NCORES) * BLK                 # rows of block x+k
            S[tgt:tgt + BLK] += cs[k - 1]
    S -= _AUX["diag"]
    lse = np.log(S)
    loss = (lse - _AUX["pos"]).mean()
    return np.array(loss, dtype=np.float32)


def kernel(feats, label, _trace=False, _repeats=1):
    global LAST_RESULT
    from concourse.bass_utils import run_bass_kernel_spmd

    nc = get_nc(_repeats)
    in_maps = make_in_maps(feats, label)
    res = run_bass_kernel_spmd(nc, in_maps, list(range(NCORES)), trace=_trace)
    LAST_RESULT = res
    return finish(res.results)


# revision 18
# speedup vs baseline: 1.1833x; 1.1833x over previous
"""Contrastive loss (SimCLR-style) on 8 Trainium2 NeuronCores.

Full inputs in, full output out.  Each core owns a 1024-row block of
feats.  The host pre-normalizes feats (fp32), casts to bf16, and passes
each core the TRANSPOSED layout nfT = nf.T [D=128, cols], rolled so the
core's own block is columns 0..1023.  Because cos = nfT.T @ nfT, the
transposed layout serves as BOTH matmul operands: no on-device norms,
no transposes, no rsqrt pipeline.

Symmetry split: exp(cos/T) is symmetric, so core x only computes its
1024 rows against local column blocks 0..4 (cols 0..5119).  Row sums
over the remaining column blocks 5..7 are recovered from *column* sums
of blocks 1..3 (ones-stationary matmuls accumulated over the 8 row
tiles), shipped to the host, which adds them into the right rows.

The self-similarity diagonal is NOT masked on device: the host knows
exactly what the device computes for z_ii (fp32 dot of the bf16 row
with itself) and subtracts exp(z_ii/T) from the assembled row sums.

exp is split across two engines: ACT evaluates blocks q=0..2 with its
Exp table (free row-sum accumulation); the DVE evaluates blocks q=3,4
with a bf16 Schraudolph bit-trick -- i16 = z*A + B truncated, bitcast
to bf16 is 2^(z*log2e/T + sigma), with B calibrated so the SUM of the
approximated exps is unbiased (row-sum rel err ~1e-3 << tolerance).
A second DVE pass row-sums both blocks per m-tile in one op.

Engine budget per core: PE does 80 similarity matmuls (5 blocks x 8
m-tiles x 2x512) plus 48 column-sum matmuls; ACT does 24 exp ops; DVE
does 16 Schraudolph ops + 8 row-sum ops + 3 column-sum copies.  Host:
normalize, transpose, positive pairs, diag subtraction, logsumexp.
"""

from contextlib import ExitStack

import numpy as np

N, D, NCORES = 8192, 128, 8
BLK = N // NCORES            # 1024 rows per core
TPB = BLK // 128             # 8 M-tiles (of 128 rows) per core
NQ = 5                       # direct column blocks per core (cols 0..5119)
NQA = 3                      # blocks on ACT (q=0..2); q=3,4 ride the DVE
CSBLKS = 3                   # column-sum blocks (local col blocks 1..3)
COLS = NQ * BLK              # 5120 columns held on device
TEMP = 0.07
EPS = 1e-8
SCH_A = float(np.float32((1.0 / TEMP) * np.log2(np.e) * 128.0))
SCH_B = 16249.0              # calibrated: unbiased sum of bf16-Schraudolph exp
NPARTS = NQ * TPB            # one row-sum column per (q, m) tile
NPAIR = 6                    # m-tiles whose q=3/q=4 row-sums share one col

_CACHE = {}
_AUX = {}
LAST_RESULT = None


def _emit(tc, xt, s_out, cs_out, rep=0):
    import concourse.mybir as mybir

    nc = tc.nc
    f32 = mybir.dt.float32
    bf16 = mybir.dt.bfloat16
    i16 = mybir.dt.int16
    AF = mybir.ActivationFunctionType
    ALU = mybir.AluOpType
    SCALE = 1.0 / TEMP

    # tiles routed to the DVE Schraudolph stream; the rest ride ACT.
    # q=3/q=4 tiles of the same m share one i16 tile so a single [128,2048]
    # row-sum op covers both (their sums land in the same output row).
    DVE_TILES = {(3, m) for m in range(NPAIR)} | {(4, m) for m in range(TPB)}

    with ExitStack() as ctx:
        singles = ctx.enter_context(tc.tile_pool(name=f"singles{rep}", bufs=1))
        esp = ctx.enter_context(tc.tile_pool(name=f"esp{rep}", bufs=TPB))
        e2p = ctx.enter_context(tc.tile_pool(name=f"e2p{rep}", bufs=16))
        junkp = ctx.enter_context(tc.tile_pool(name=f"junkp{rep}", bufs=2))

        xbig = singles.tile([128, COLS], bf16, tag="xbig")    # rolled nf^T
        onesb = singles.tile([128, 128], bf16, tag="onesb")
        parts = singles.tile([128, NPARTS], f32, tag="parts")
        colacc = singles.tile([1, CSBLKS * BLK], f32, tag="colacc")

        # block 0 first (stationaries + q=0 moving), rest streams in under
        # the q=0 round's compute
        nc.sync.dma_start(out=xbig[:, 0:BLK], in_=xt[:, 0:BLK])
        nc.sync.dma_start(out=xbig[:, BLK:COLS], in_=xt[:, BLK:COLS])
        nc.vector.memset(onesb[:], 1.0)
        nc.vector.memset(parts[:], 0.0)

        # prime the ACT Exp table at t~0 so the first real exp doesn't pay
        # the table load on the critical path
        warm = singles.tile([128, 1], f32, tag="warm")
        nc.vector.memset(warm[:], 0.0)
        nc.scalar.activation(warm[:], warm[:], AF.Exp)

        with (
            tc.tile_pool(name=f"mpsum{rep}", bufs=3, space="PSUM") as mpsum,
            tc.tile_pool(name=f"cpsum{rep}", bufs=1, space="PSUM") as cpsum,
        ):
            # Interleave the two consumer streams so ACT and DVE run
            # CONCURRENTLY (a q-outer loop would phase-separate them: PSUM
            # buffering forces production order == consumption order).
            # (3,7) leads the ACT list so the q=3 column-sum batch can fire
            # early instead of trailing the kernel.
            act_list = [(3, m) for m in range(NPAIR, TPB)] + \
                [(q, m) for q in range(NQA) for m in range(TPB)]
            dve_list = sorted(DVE_TILES)
            sched = []
            ai = di = 0
            while ai < len(act_list) or di < len(dve_list):
                if ai * len(dve_list) <= di * len(act_list):
                    if ai < len(act_list):
                        sched.append(("A",) + act_list[ai]); ai += 1
                    else:
                        sched.append(("D",) + dve_list[di]); di += 1
                else:
                    if di < len(dve_list):
                        sched.append(("D",) + dve_list[di]); di += 1
                    else:
                        sched.append(("A",) + act_list[ai]); ai += 1

            etile_of = {}
            e2tiles = [None] * NPAIR
            done_count = {q: 0 for q in range(NQ)}

            def emit_cs(q):
                # column sums of exp accumulated across the 8 M-tiles;
                # one ones-LDWEIGHTS per batch
                cs = cpsum.tile([128, BLK], f32, name="cs", tag="cs")
                for m in range(TPB):
                    eap = etile_of[(q, m)]
                    nc.tensor.matmul(cs[:, 0:512], onesb[:], eap[:, 0:512],
                                     start=(m == 0), stop=(m == TPB - 1),
                                     skip_group_check=True)
                    nc.tensor.matmul(cs[:, 512:1024], onesb[:],
                                     eap[:, 512:1024],
                                     start=(m == 0), stop=(m == TPB - 1),
                                     skip_group_check=True)
                k = q - 1
                nc.scalar.copy(colacc[0:1, k * BLK:(k + 1) * BLK], cs[0:1, :])

            for stream, q, m in sched:
                z = mpsum.tile([128, BLK], f32, name="z", tag="z")
                lhsT = xbig[:, m * 128:(m + 1) * 128]
                c0 = q * BLK
                nc.tensor.matmul(z[:, 0:512], lhsT, xbig[:, c0:c0 + 512])
                nc.tensor.matmul(z[:, 512:1024], lhsT,
                                 xbig[:, c0 + 512:c0 + 1024])
                col = q * TPB + m
                if stream == "A":
                    # ACT stream: spline exp, free row-sum accumulation
                    e = esp.tile([128, BLK], bf16, name="e", tag="e")
                    nc.scalar.activation(e[:], z[:], AF.Exp, scale=SCALE,
                                         accum_out=parts[:, col:col + 1])
                    etile_of[(q, m)] = e[:]
                else:
                    # DVE stream: i16 = z*A + B, bitcast bf16 ~ exp(z/T)
                    paired = m < NPAIR
                    if paired and q == NQA:
                        e2tiles[m] = e2p.tile([128, 2 * BLK], i16,
                                              name="e2", tag="e2")
                    if paired:
                        es = e2tiles[m][:, (q - NQA) * BLK:
                                        (q - NQA + 1) * BLK]
                    else:
                        est = e2p.tile([128, BLK], i16, name="est", tag="est")
                        es = est[:]
                    nc.vector.tensor_scalar(es, z[:], SCH_A, SCH_B,
                                            ALU.mult, ALU.add)
                    etile_of[(q, m)] = es.bitcast(bf16)
                    if paired and q == NQ - 1:
                        # one [128,2048] row-sum covering q=3 and q=4
                        eb = e2tiles[m][:].bitcast(bf16)
                        j = junkp.tile([128, 2 * BLK], bf16, name="j",
                                       tag="junk")
                        nc.vector.tensor_scalar(
                            j[:], eb, 1.0, 0.0, ALU.mult, ALU.add,
                            accum_out=parts[:, NQA * TPB + m:
                                            NQA * TPB + m + 1])
                    elif not paired:
                        jb = junkp.tile([128, BLK], bf16, name="jb",
                                        tag="junkb")
                        nc.vector.tensor_scalar(
                            jb[:], es.bitcast(bf16), 1.0, 0.0,
                            ALU.mult, ALU.add,
                            accum_out=parts[:, col:col + 1])
                done_count[q] += 1
                if 1 <= q <= CSBLKS and done_count[q] == TPB:
                    emit_cs(q)

        nc.sync.dma_start(out=s_out, in_=parts[:])
        nc.sync.dma_start(out=cs_out, in_=colacc[:])


def declare_io(nc):
    """Declare the kernel's DRAM I/O on `nc`; returns the APs _emit wants."""
    import concourse.mybir as mybir

    f32 = mybir.dt.float32
    bf16 = mybir.dt.bfloat16
    xt_h = nc.dram_tensor("xt", [128, COLS], bf16, kind="ExternalInput")
    s_h = nc.dram_tensor("s_out", [128, NPARTS], f32, kind="ExternalOutput")
    c_h = nc.dram_tensor("cs_out", [1, CSBLKS * BLK], f32,
                         kind="ExternalOutput")
    return (xt_h.ap(), s_h.ap(), c_h.ap())


def _build_nc(repeats=1):
    import concourse.tile as tile
    from concourse import bacc

    nc = bacc.Bacc(
        "TRN2", target_bir_lowering=False, debug=False,
        enable_asserts=False, num_devices=NCORES,
    )
    aps = declare_io(nc)
    with tile.TileContext(nc, trace_sim=False) as tc:
        for rep in range(repeats):
            _emit(tc, *aps, rep=rep)
    nc.compile()
    return nc


def get_nc(repeats=1):
    key = ("nc", repeats)
    if key not in _CACHE:
        _CACHE[key] = _build_nc(repeats)
    return _CACHE[key]


def make_in_maps(feats, label):
    import ml_dtypes

    feats = np.asarray(feats, dtype=np.float32)
    label = np.asarray(label)
    norms = np.sqrt((feats ** 2).sum(axis=1))
    nf = feats / np.maximum(norms, EPS)[:, None]
    nf16 = nf.astype(ml_dtypes.bfloat16)
    nfT = np.ascontiguousarray(nf16.T)                 # [128, 8192]
    nfT2 = np.concatenate([nfT, nfT], axis=1)          # wraparound roll
    in_maps = [
        {"xt": np.ascontiguousarray(nfT2[:, c * BLK:c * BLK + COLS])}
        for c in range(NCORES)
    ]

    # host-side replication of the device's self-similarity term:
    # z_ii = fp32 dot of the bf16 row with itself, e_ii = exp(z_ii * 1/T)
    nf16f = nf16.astype(np.float32)
    ssq = (nf16f ** 2).sum(axis=1, dtype=np.float32)
    _AUX["diag"] = np.exp((ssq * np.float32(1.0 / TEMP)).astype(np.float32)
                          ).astype(np.float64)
    pos_idx = np.argmax(label, axis=1)
    nf64 = nf.astype(np.float64)
    _AUX["pos"] = (nf64 * nf64[pos_idx]).sum(axis=1) / TEMP
    return in_maps


def finish(results):
    """Host epilogue: assemble full row sums from direct row partials and
    symmetric column partials, subtract the exact diagonal, logsumexp,
    subtract positive-pair logits, mean."""
    S = np.zeros(N, dtype=np.float64)
    for x in range(NCORES):
        parts = results[x]["s_out"].astype(np.float64)     # [128, NPARTS]
        pv = parts.reshape(128, NQ, TPB)
        # cols q=0..3 always used (col 3*8+m holds q3+q4 for paired m);
        # q=4 cols only for the unpaired m-tiles
        sv = pv[:, :4, :].sum(axis=1)                      # [128, TPB]
        sv[:, NPAIR:] += pv[:, 4, NPAIR:]
        S[x * BLK:(x + 1) * BLK] += sv.T.reshape(-1)       # row = m*128+p
        cs = results[x]["cs_out"].astype(np.float64).reshape(CSBLKS, BLK)
        for k in range(1, CSBLKS + 1):
            tgt = ((x + k) % NCORES) * BLK                 # rows of block x+k
            S[tgt:tgt + BLK] += cs[k - 1]
    S -= _AUX["diag"]
    lse = np.log(S)
    loss = (lse - _AUX["pos"]).mean()
    return np.array(loss, dtype=np.float32)


def kernel(feats, label, _trace=False, _repeats=1):
    global LAST_RESULT
    from concourse.bass_utils import run_bass_kernel_spmd

    nc = get_nc(_repeats)
    in_maps = make_in_maps(feats, label)
    res = run_bass_kernel_spmd(nc, in_maps, list(range(NCORES)), trace=_trace)
    LAST_RESULT = res
    return finish(res.results)


# revision 24
# speedup vs baseline: 1.3990x; 1.1823x over previous
"""Contrastive loss (SimCLR-style) on 8 Trainium2 NeuronCores.

Full inputs in, full output out.  Each core owns a 1024-row block of
feats.  The host pre-normalizes feats (fp32), casts to bf16, and passes
each core the TRANSPOSED layout nfT = nf.T [D=128, cols], rolled so the
core's own block is columns 0..1023.  Because cos = nfT.T @ nfT, the
transposed layout serves as BOTH matmul operands: no on-device norms,
no transposes, no rsqrt pipeline.

Symmetry split: exp(cos/T) is symmetric, so core x only computes its
1024 rows against local column blocks 0..4 (cols 0..5119).  Row sums
over the remaining column blocks 5..7 are recovered from *column* sums
of blocks 1..3 (ones-stationary matmuls accumulated over the 8 row
tiles), shipped to the host, which adds them into the right rows.

The self-similarity diagonal is NOT masked on device: the host knows
exactly what the device computes for z_ii (fp32 dot of the bf16 row
with itself) and subtracts exp(z_ii/T) from the assembled row sums.

Engine budget per core: PE does 80 similarity matmuls (5 blocks x 8
m-tiles x 2x512) plus 48 column-sum matmuls; ACT does all 40 exp ops
([128,1024] PSUM->SBUF, free row-sum accumulation); DVE only copies the
3 column-sum rows out of PSUM.  Host: normalize, transpose, positive
pairs, diag subtraction, logsumexp, mean.
"""

from contextlib import ExitStack

import numpy as np

N, D, NCORES = 8192, 128, 8
BLK = N // NCORES            # 1024 rows per core
TPB = BLK // 128             # 8 M-tiles (of 128 rows) per core
NQ = 5                       # direct column blocks per core (cols 0..5119)
CSBLKS = 3                   # column-sum blocks (local col blocks 1..3)
COLS = NQ * BLK              # 5120 columns held on device
TEMP = 0.07
EPS = 1e-8
SCH_A = float(np.float32((1.0 / TEMP) * np.log2(np.e) * 128.0))
SCH_B = 16249.0              # calibrated: unbiased sum of bf16-Schraudolph exp
PAIR_MS = (2, 5, 7)          # m-tiles of q=1/q=2 routed to the DVE stream
SINGLE_MS = (3, 6)           # m-tiles of q=3 routed to the DVE stream

_CACHE = {}
_AUX = {}
LAST_RESULT = None


def _emit(tc, xt, s_out, cs_out, rep=0):
    import concourse.mybir as mybir

    nc = tc.nc
    f32 = mybir.dt.float32
    bf16 = mybir.dt.bfloat16
    i16 = mybir.dt.int16
    AF = mybir.ActivationFunctionType
    ALU = mybir.AluOpType
    SCALE = 1.0 / TEMP

    with ExitStack() as ctx:
        singles = ctx.enter_context(tc.tile_pool(name=f"singles{rep}", bufs=1))
        esp = ctx.enter_context(tc.tile_pool(name=f"esp{rep}", bufs=12))
        e2p = ctx.enter_context(tc.tile_pool(name=f"e2p{rep}", bufs=6))
        junkp = ctx.enter_context(tc.tile_pool(name=f"junkp{rep}", bufs=2))

        xbig = singles.tile([128, COLS], bf16, tag="xbig")    # rolled nf^T
        onesb = singles.tile([128, 128], bf16, tag="onesb")
        parts = singles.tile([128, NQ * TPB], f32, tag="parts")
        colacc = singles.tile([1, CSBLKS * BLK], f32, tag="colacc")

        # block 0 first (stationaries), then the cs blocks 1..3, then block
        # 4 (only needed by the final paired q0+q4 phase)
        nc.sync.dma_start(out=xbig[:, 0:BLK], in_=xt[:, 0:BLK])
        nc.sync.dma_start(out=xbig[:, BLK:2 * BLK], in_=xt[:, BLK:2 * BLK])
        nc.sync.dma_start(out=xbig[:, 2 * BLK:COLS], in_=xt[:, 2 * BLK:COLS])
        nc.vector.memset(onesb[:], 1.0)
        nc.vector.memset(parts[:], 0.0)

        # prime the ACT Exp table at t~0 so the first real exp doesn't pay
        # the table load on the critical path
        warm = singles.tile([128, 1], f32, tag="warm")
        nc.vector.memset(warm[:], 0.0)
        nc.scalar.activation(warm[:], warm[:], AF.Exp)

        # --- phase 1: the three cs blocks (q=1..3), q-outer so the column
        # sums accumulate across the 8 M-tiles in 2 PSUM banks.  A few
        # tiles per round ride a DVE Schraudolph stream (i16 = z*A + B,
        # bitcast bf16 ~ exp(z/T)) so ACT and DVE work concurrently;
        # q=1/q=2 tiles of equal m share one i16 tile so a single
        # [128,2048] row-sum op covers both (same output rows).
        pair_e2 = {}
        with (
            tc.tile_pool(name=f"mpsum{rep}", bufs=3, space="PSUM") as mpsum,
            tc.tile_pool(name=f"cpsum{rep}", bufs=1, space="PSUM") as cpsum,
        ):
            for q in range(1, CSBLKS + 1):
                etiles = []
                for m in range(TPB):
                    z = mpsum.tile([128, BLK], f32, name="z", tag="z")
                    lhsT = xbig[:, m * 128:(m + 1) * 128]
                    c0 = q * BLK
                    nc.tensor.matmul(z[:, 0:512], lhsT, xbig[:, c0:c0 + 512])
                    nc.tensor.matmul(z[:, 512:1024], lhsT,
                                     xbig[:, c0 + 512:c0 + 1024])
                    col = q * TPB + m
                    on_dve = (q in (1, 2) and m in PAIR_MS) or \
                             (q == 3 and m in SINGLE_MS)
                    if not on_dve:
                        e = esp.tile([128, BLK], bf16, name="e", tag="e")
                        nc.scalar.activation(e[:], z[:], AF.Exp, scale=SCALE,
                                             accum_out=parts[:, col:col + 1])
                        etiles.append(e[:])
                    elif q == 1:
                        e2 = e2p.tile([128, 2 * BLK], i16, name="e2",
                                      tag="e2")
                        pair_e2[m] = e2
                        nc.vector.tensor_scalar(e2[:, 0:BLK], z[:], SCH_A,
                                                SCH_B, ALU.mult, ALU.add)
                        etiles.append(e2[:, 0:BLK].bitcast(bf16))
                    elif q == 2:
                        e2 = pair_e2[m]
                        nc.vector.tensor_scalar(e2[:, BLK:2 * BLK], z[:],
                                                SCH_A, SCH_B, ALU.mult,
                                                ALU.add)
                        etiles.append(e2[:, BLK:2 * BLK].bitcast(bf16))
                        # one [128,2048] row-sum covering q=1 and q=2;
                        # lands in q=1's column (q=2's stays zero)
                        j = junkp.tile([128, 2 * BLK], bf16, name="j",
                                       tag="junk")
                        nc.vector.tensor_scalar(
                            j[:], e2[:].bitcast(bf16), 1.0, 0.0,
                            ALU.mult, ALU.add,
                            accum_out=parts[:, TPB + m:TPB + m + 1])
                    else:
                        es = e2p.tile([128, BLK], i16, name="es", tag="es")
                        nc.vector.tensor_scalar(es[:], z[:], SCH_A, SCH_B,
                                                ALU.mult, ALU.add)
                        etiles.append(es[:].bitcast(bf16))
                        jb = junkp.tile([128, BLK], bf16, name="jb",
                                        tag="junkb")
                        nc.vector.tensor_scalar(
                            jb[:], es[:].bitcast(bf16), 1.0, 0.0,
                            ALU.mult, ALU.add,
                            accum_out=parts[:, col:col + 1])
                # column sums of exp accumulated across the 8 M-tiles;
                # one ones-LDWEIGHTS per round (batched after the e's)
                cs = cpsum.tile([128, BLK], f32, name="cs", tag="cs")
                for m in range(TPB):
                    nc.tensor.matmul(cs[:, 0:512], onesb[:],
                                     etiles[m][:, 0:512],
                                     start=(m == 0), stop=(m == TPB - 1),
                                     skip_group_check=True)
                    nc.tensor.matmul(cs[:, 512:1024], onesb[:],
                                     etiles[m][:, 512:1024],
                                     start=(m == 0), stop=(m == TPB - 1),
                                     skip_group_check=True)
                k = q - 1
                nc.vector.tensor_copy(colacc[0:1, k * BLK:(k + 1) * BLK],
                                      cs[0:1, :])

        # --- phase 2: q=0 and q=4 paired per m-tile into one [128,2048]
        # PSUM tile (4 banks) consumed by a single ACT exp whose free
        # accumulation sums BOTH blocks for the same rows (halves the
        # per-op overhead); one LDWEIGHTS serves all 4 matmuls.
        with tc.tile_pool(name=f"ppsum{rep}", bufs=2, space="PSUM") as ppsum:
            for m in range(TPB):
                zp = ppsum.tile([128, 2 * BLK], f32, name="zp", tag="zp")
                lhsT = xbig[:, m * 128:(m + 1) * 128]
                c4 = 4 * BLK
                nc.tensor.matmul(zp[:, 0:512], lhsT, xbig[:, 0:512])
                nc.tensor.matmul(zp[:, 512:1024], lhsT, xbig[:, 512:1024])
                nc.tensor.matmul(zp[:, 1024:1536], lhsT,
                                 xbig[:, c4:c4 + 512])
                nc.tensor.matmul(zp[:, 1536:2048], lhsT,
                                 xbig[:, c4 + 512:c4 + 1024])
                ep = esp.tile([128, 2 * BLK], bf16, name="ep", tag="ep")
                nc.scalar.activation(ep[:], zp[:], AF.Exp, scale=SCALE,
                                     accum_out=parts[:, m:m + 1])

        nc.sync.dma_start(out=s_out, in_=parts[:])
        nc.sync.dma_start(out=cs_out, in_=colacc[:])


def declare_io(nc):
    """Declare the kernel's DRAM I/O on `nc`; returns the APs _emit wants."""
    import concourse.mybir as mybir

    f32 = mybir.dt.float32
    bf16 = mybir.dt.bfloat16
    xt_h = nc.dram_tensor("xt", [128, COLS], bf16, kind="ExternalInput")
    s_h = nc.dram_tensor("s_out", [128, NQ * TPB], f32, kind="ExternalOutput")
    c_h = nc.dram_tensor("cs_out", [1, CSBLKS * BLK], f32,
                         kind="ExternalOutput")
    return (xt_h.ap(), s_h.ap(), c_h.ap())


def _build_nc(repeats=1):
    import concourse.tile as tile
    from concourse import bacc

    nc = bacc.Bacc(
        "TRN2", target_bir_lowering=False, debug=False,
        enable_asserts=False, num_devices=NCORES,
    )
    aps = declare_io(nc)
    with tile.TileContext(nc, trace_sim=False) as tc:
        for rep in range(repeats):
            _emit(tc, *aps, rep=rep)
    nc.compile()
    return nc


def get_nc(repeats=1):
    key = ("nc", repeats)
    if key not in _CACHE:
        _CACHE[key] = _build_nc(repeats)
    return _CACHE[key]


def make_in_maps(feats, label):
    import ml_dtypes

    feats = np.asarray(feats, dtype=np.float32)
    label = np.asarray(label)
    norms = np.sqrt((feats ** 2).sum(axis=1))
    nf = feats / np.maximum(norms, EPS)[:, None]
    nf16 = nf.astype(ml_dtypes.bfloat16)
    nfT = np.ascontiguousarray(nf16.T)                 # [128, 8192]
    nfT2 = np.concatenate([nfT, nfT], axis=1)          # wraparound roll
    in_maps = [
        {"xt": np.ascontiguousarray(nfT2[:, c * BLK:c * BLK + COLS])}
        for c in range(NCORES)
    ]

    # host-side replication of the device's self-similarity term:
    # z_ii = fp32 dot of the bf16 row with itself, e_ii = exp(z_ii * 1/T)
    nf16f = nf16.astype(np.float32)
    ssq = (nf16f ** 2).sum(axis=1, dtype=np.float32)
    _AUX["diag"] = np.exp((ssq * np.float32(1.0 / TEMP)).astype(np.float32)
                          ).astype(np.float64)
    pos_idx = np.argmax(label, axis=1)
    nf64 = nf.astype(np.float64)
    _AUX["pos"] = (nf64 * nf64[pos_idx]).sum(axis=1) / TEMP
    return in_maps


def finish(results):
    """Host epilogue: assemble full row sums from direct row partials and
    symmetric column partials, subtract the exact diagonal, logsumexp,
    subtract positive-pair logits, mean."""
    S = np.zeros(N, dtype=np.float64)
    for x in range(NCORES):
        parts = results[x]["s_out"].astype(np.float64)     # [128, NQ*TPB]
        # merged columns: q0 col m holds q0+q4, q1 col m holds q1+q2 for
        # DVE-paired m's; every unused column is memset-zero on device,
        # so a plain sum over q is correct.
        sv = parts.reshape(128, NQ, TPB).sum(axis=1)       # [128, TPB]
        S[x * BLK:(x + 1) * BLK] += sv.T.reshape(-1)       # row = m*128+p
        cs = results[x]["cs_out"].astype(np.float64).reshape(CSBLKS, BLK)
        for k in range(1, CSBLKS + 1):
            tgt = ((x + k) % NCORES) * BLK                 # rows of block x+k
            S[tgt:tgt + BLK] += cs[k - 1]
    S -= _AUX["diag"]
    lse = np.log(S)
    loss = (lse - _AUX["pos"]).mean()
    return np.array(loss, dtype=np.float32)


def kernel(feats, label, _trace=False, _repeats=1):
    global LAST_RESULT
    from concourse.bass_utils import run_bass_kernel_spmd

    nc = get_nc(_repeats)
    in_maps = make_in_maps(feats, label)
    res = run_bass_kernel_spmd(nc, in_maps, list(range(NCORES)), trace=_trace)
    LAST_RESULT = res
    return finish(res.results)
